# revision 9
# baseline (speedup 1.0000x reference)
"""ColorUnpool (gather + segment-max + relu) as an 8-core Trainium2 Bass kernel.

Reference semantics:
    out = zeros([200000, 256]);  out[center_idx] = feat            # centers
    seg = segment_max(feat[edge_src], edge_dst)                    # edges
    out[r] = max(seg[r], 0) for rows r with >= 1 incoming edge

edge_dst only hits rows [50000, 200000) and center_idx only [0, 50000), so
the two regions are disjoint.  The center region is a pure host-side copy of
the input (no compute); the device computes the edge region only.

Device strategy (per core, dst rows split 8 ways -> 18750 rows/core):
  * Rows are degree-sorted (desc) and packed into 147 tiles of 128 rows,
    processed in bands of 32 tiles.  Within a band the column layout is
    round-major: round 0 holds one column per tile (edge 0 of every row,
    ZID pad for deg-0 rows); round j>=1 holds a column per band tile whose
    max degree exceeds j (a prefix, tiles being degree-sorted).
  * The feat table is compacted per core to its ~31.6k distinct src rows
    (< 32768), so gather indices fit in int16 and the gather runs as
    1024-index `dma_gather` instructions (the HW cap) round-robined over
    all 4 SWDGE queues -- descriptor generation for different queues runs
    concurrently on the Q7 cores, which quarters the ~7.7ns/row software
    DGE cost that serialized the old per-column indirect-DMA design.
  * Round 0 gathers straight into the accumulator; rounds j>=1 gather into
    rotating SBUF chunks and fold in with fused DVE ops
    acc = max(max(acc, 0), g)  (scalar_tensor_tensor), which also bakes in
    the final relu.  Tiles only touched by round 0 get an Activation-engine
    relu instead.  Each band is written back to DRAM as soon as its last
    round completes, overlapping output DMA with the remaining gathers;
    the last band is all degree<=1 tiles, so the post-gather tail is tiny.
  * The idx plane is loaded in per-band pieces so the first gather starts
    as soon as the first piece lands.
  * feat is bf16 on device (rel err ~4e-3 << 2e-2 gate); the host
    un-permutes rows and upcasts to f32.
"""

import sys
import types

import numpy as np
import ml_dtypes

sys.path.insert(0, "/opt/trn_rl_repo")

N_NODES = 200000
N_CENTERS = 50000
FEAT = 256
NCORES = 8
P = 128

R_EDGE = N_NODES - N_CENTERS          # 150000 edge-target rows
RC = R_EDGE // NCORES                 # 18750 edge rows per core
TILES = (RC + P - 1) // P             # 147 tiles of 128 rows
NPOS = TILES * P                      # 18816 padded row slots
TBL = 32768                           # per-core compact feat table rows
ZID = TBL - 1                         # zero row id (table is zero-padded)
G = 8                                 # gather chunk width (cols); HW caps a
                                      # single dma_gather at 1024 indices
BAND = 32                             # tiles per band


def _install_profile_hook():
    """Provide antenv.axon_hooks (missing on this image) so that
    run_bass_kernel_spmd(trace=True) can profile via the axon .so."""
    try:
        import antenv
        if "antenv.axon_hooks" in sys.modules:
            return
        from trn_agent_boot.trn_boot import _ntff_profile_via_ctypes
        mod = types.ModuleType("antenv.axon_hooks")
        hook = _ntff_profile_via_ctypes("/opt/axon/libaxon_pjrt.so")
        mod.get_axon_ntff_profile_hook = lambda: hook
        mod.set_axon_ntff_profile_hook = lambda h: None
        sys.modules["antenv.axon_hooks"] = mod
        antenv.axon_hooks = mod
    except Exception:
        pass


def _layout(d_un):
    """Column layout from the union per-tile max degrees.

    Returns (cols, bands) where cols is a list of (band, j, t0, ntiles)
    segments in emission order (each segment's columns are tiles
    [t0, t0+ntiles) of round j) and bands is a list of
    (t0, t1, seg_lo, seg_hi, relu_t0): tile range, segment index range and
    the first tile needing an explicit relu (degree <= 1).
    """
    cols = []
    bands = []
    for t0 in range(0, TILES, BAND):
        t1 = min(t0 + BAND, TILES)
        seg_lo = len(cols)
        cols.append((len(bands), 0, t0, t1 - t0))            # round 0
        maxd = int(d_un[t0])                                 # desc sorted
        for j in range(1, maxd):
            n = int((d_un[t0:t1] > j).sum())
            if n > 0:
                cols.append((len(bands), j, t0, n))
        relu_t0 = t0 + int((d_un[t0:t1] > 1).sum())
        bands.append((t0, t1, seg_lo, len(cols), relu_t0))
    return cols, bands


def _build_plan(edge_src, edge_dst, feat):
    """Host preprocessing.

    Returns (cols, bands, C, col_base, tables, idx_planes, orders).
    """
    edge_src = np.asarray(edge_src, np.int64)
    edge_dst = np.asarray(edge_dst, np.int64)
    local_dst = edge_dst - N_CENTERS
    assert local_dst.min() >= 0 and local_dst.max() < R_EDGE
    core_of = local_dst // RC

    percore = []
    d_un = np.zeros(TILES, np.int64)
    for c in range(NCORES):
        m = core_of == c
        ld = (local_dst[m] % RC).astype(np.int64)
        ss = edge_src[m].astype(np.int64)
        deg = np.bincount(ld, minlength=RC)
        order = np.argsort(-deg, kind="stable")          # rows desc by degree
        eo = np.argsort(ld, kind="stable")
        ss_sorted = ss[eo]                               # CSR values
        starts = np.concatenate([[0], np.cumsum(deg)[:-1]])
        uniq, inv = np.unique(ss_sorted, return_inverse=True)
        assert len(uniq) < TBL, f"core {c}: {len(uniq)} distinct srcs > int16"
        ssc = inv.astype(np.int64)                       # compact CSR values
        deg_sorted = deg[order]
        d_tile = deg_sorted[np.arange(TILES) * P]        # per-tile max degree
        d_un = np.maximum(d_un, d_tile)
        percore.append(dict(deg=deg, order=order, ssc=ssc, starts=starts,
                            uniq=uniq))

    cols, bands = _layout(d_un)
    col_base = np.concatenate([[0], np.cumsum([n for _, _, _, n in cols])])
    C = int(col_base[-1])

    tables, idx_planes, orders = [], [], []
    for pc in percore:
        order_padded = np.full(NPOS, -1, np.int64)
        order_padded[:RC] = pc["order"]
        deg, starts, ssc = pc["deg"], pc["starts"], pc["ssc"]
        vals = np.full(C * P, ZID, np.int64)
        for si, (_, j, t0, n) in enumerate(cols):
            qpos = np.arange(t0 * P, (t0 + n) * P)
            r = order_padded[qpos]
            rs = np.where(r >= 0, r, 0)
            has = (r >= 0) & (deg[rs] > j)
            v = np.where(has, ssc[np.minimum(starts[rs] + j, len(ssc) - 1)],
                         ZID)
            base = col_base[si] * P
            vals[base:base + n * P] = v
        # idx position g lives at [g%16, g//16], replicated x8 for Q7 cores
        plane16 = vals.astype(np.int16).reshape(C * 8, 16).T
        idx_planes.append(np.ascontiguousarray(np.tile(plane16, (8, 1))))
        tbl = np.zeros((TBL, FEAT), ml_dtypes.bfloat16)
        tbl[:len(pc["uniq"])] = feat[pc["uniq"]].astype(ml_dtypes.bfloat16)
        tables.append(tbl)
        orders.append(pc["order"])
    return cols, bands, C, col_base, tables, idx_planes, orders


def _build_bass(cols, bands, C, col_base):
    import concourse.bacc as bacc
    import concourse.mybir as mybir
    import concourse.tile as tile

    nc = bacc.Bacc("TRN2", target_bir_lowering=False, debug=False,
                   num_devices=NCORES, num_swdge_queues=4)
    t_feat = nc.dram_tensor("feat_tbl", [TBL, FEAT], mybir.dt.bfloat16,
                            kind="ExternalInput")
    t_idx = nc.dram_tensor("idxs", [P, C * 8], mybir.dt.int16,
                           kind="ExternalInput")
    t_oe = nc.dram_tensor("out_edge", [P, TILES, FEAT], mybir.dt.bfloat16,
                          kind="ExternalOutput")

    mx = mybir.AluOpType.max
    relu = mybir.ActivationFunctionType.Relu
    qn = 0

    with tile.TileContext(nc) as tc:
        with tc.tile_pool(name="idxp", bufs=1) as idxp, \
             tc.tile_pool(name="accp", bufs=1) as accp, \
             tc.tile_pool(name="gp", bufs=8) as gp:
            idx = idxp.tile([P, C * 8], mybir.dt.int16)
            # idx plane loaded per band so gather 0 starts early; band 0
            # split again in two
            loads = [(int(col_base[s_lo]), int(col_base[s_hi]))
                     for _, _, s_lo, s_hi, _ in bands]
            l0, l1 = loads[0]
            loads[0:1] = [(l0, (l0 + l1) // 2), ((l0 + l1) // 2, l1)]
            for a, b in loads:
                nc.sync.dma_start(out=idx[:, a * 8:b * 8],
                                  in_=t_idx[:, a * 8:b * 8])
            acc = accp.tile([P, TILES, FEAT], mybir.dt.bfloat16)

            for t0, t1, s_lo, s_hi, relu_t0 in bands:
                # chunk the band's columns; acc-direct round-0 chunks first
                r0_lo, r0_hi = int(col_base[s_lo]), int(col_base[s_lo + 1])
                j_lo, j_hi = r0_hi, int(col_base[s_hi])
                chunks = [(s, min(s + G, r0_hi), True)
                          for s in range(r0_lo, r0_hi, G)]
                chunks += [(s, min(s + G, j_hi), False)
                           for s in range(j_lo, j_hi, G)]
                for cs, ce, direct in chunks:
                    w = ce - cs
                    if direct:
                        gout = acc[:, t0 + (cs - r0_lo):t0 + (ce - r0_lo), :]
                    else:
                        g = gp.tile([P, G, FEAT], mybir.dt.bfloat16, tag="g")
                        gout = g[:, :w, :]
                    nc.gpsimd.dma_gather(gout, t_feat[:],
                                         idx[:, cs * 8:ce * 8],
                                         w * P, w * P, FEAT, queue_num=qn)
                    qn = (qn + 1) % 4
                    if direct:
                        continue
                    # fused max+relu pieces for the rounds this chunk covers
                    for si in range(s_lo + 1, s_hi):
                        a = max(cs, int(col_base[si]))
                        b = min(ce, int(col_base[si + 1]))
                        if a < b:
                            tt = t0 + (a - int(col_base[si]))
                            L = b - a
                            nc.vector.scalar_tensor_tensor(
                                out=acc[:, tt:tt + L, :],
                                in0=acc[:, tt:tt + L, :], scalar=0.0,
                                in1=g[:, a - cs:b - cs, :], op0=mx, op1=mx)
                if relu_t0 < t1:
                    # degree<=1 tiles: relu never fused -> Act engine
                    nc.scalar.activation(acc[:, relu_t0:t1, :],
                                         acc[:, relu_t0:t1, :], relu)
                nc.sync.dma_start(out=t_oe[:, t0:t1, :], in_=acc[:, t0:t1, :])
    nc.compile()
    return nc


def _unshard(results, orders, feat_centers):
    out = np.empty((N_NODES, FEAT), np.float32)
    out[:N_CENTERS] = feat_centers                       # centers: exact copy
    for c in range(NCORES):
        oe = np.asarray(results[c]["out_edge"])          # [P, TILES, FEAT]
        vals = oe.transpose(1, 0, 2).reshape(NPOS, FEAT)  # position-major
        rows = N_CENTERS + c * RC + orders[c]            # position q -> row
        out[rows] = vals[:RC].astype(np.float32)
    return out


def kernel(feat, center_idx, edge_src, edge_dst, n_nodes, _trace=False):
    assert int(n_nodes) == N_NODES
    feat = np.ascontiguousarray(np.asarray(feat, np.float32))
    center_idx = np.asarray(center_idx, np.int64)

    # centers: out[center_idx] = feat, handled fully on the host (pure copy)
    feat_centers = np.zeros((N_CENTERS, FEAT), np.float32)
    feat_centers[center_idx] = feat

    cols, bands, C, col_base, tables, idx_planes, orders = _build_plan(
        edge_src, edge_dst, feat)
    nc = _build_bass(cols, bands, C, col_base)

    if _trace:
        _install_profile_hook()
    import concourse.bass_utils as bass_utils
    bass_utils.upload_artifacts = lambda tmpdir: f"file://{tmpdir}"
    from concourse.bass_utils import run_bass_kernel_spmd

    in_maps = [{"feat_tbl": tables[c], "idxs": idx_planes[c]}
               for c in range(NCORES)]
    kw = dict(trace=True) if _trace else {}
    res = run_bass_kernel_spmd(nc, in_maps, list(range(NCORES)), **kw)

    out = _unshard(res.results, orders, feat_centers)
    if _trace:
        return out, res
    return out
